# revision 1
# baseline (speedup 1.0000x reference)
"""Multi-head attention Bass kernel v2 for Trainium2, SPMD over 8 NeuronCores.

Problem: q,k,v [4, 16, 2048, 64] fp32 -> softmax(q@k^T/sqrt(64))@v.
64 (batch*head) heads, 8 consecutive heads per core, no cross-core
communication.

Host precasts inputs to f16 (q prescaled by 2^-8, exact) so on-device
scores are s_true/32.  Per-core per-head algorithm (N=2048, D=64):

  - Q^T,K^T [128, 1024] f16 land directly via DMA-xbar-transpose of the
    [1024, 128] head view: out partitions 0:64 = d of even n, 64:128 =
    d of odd n; columns = n/2.  kTswap (parity-swapped copy, via two
    SBUF->SBUF DMAs) lets every (q-parity, k-parity) S matmul use lhsT
    and rhs on the same partition range.
  - V is DMA'd directly into vaug [128, 16*65] f16 slots (k-tiles 0-7 =
    even k, 8-15 = odd k); column 64 of each 65-block is memset to 1.0
    so the PV matmul also accumulates the softmax denominator.
  - 32 S-steps per head (parity qh x k-tile kt): S^T tile [128, 1024]
    f32 PSUM (2 f16 matmuls, contraction d=64).
  - exp on ACT (exp(32*s), f16 out) or on DVE (2-op cubic+squarings
    custom op, f16 intermediate), split ~22/10 to balance engines.
  - PV natural orientation: O[q-tile] [128, 65] += P^T-block.T @
    vaug-block, f32 PSUM, accumulated over the 16 k-tiles.  Streams 65
    rows per matmul (vs 128 the transposed orientation would need).
  - Drain per parity on GPSIMD: PSUM->SBUF copy + normalize_recip
    (divide by the ones-column) -> ostage -> one DMA per parity.

PSUM: 3x[128,1024] S ring (6 banks) + 2 shared O banks.
"""

import numpy as np

B, H, N, D = 4, 16, 2048, 64
NCORES = 8
HEADS = B * H          # 64
HPC = HEADS // NCORES  # 8 heads per core
NT = 16                # k tiles of 128 rows (0-7 even k, 8-15 odd k)
NSTEP = 32             # steps per head = 2 parities x 16 k-tiles
PRESCALE = 0.125 / 32.0   # folded into host-side q cast; exact 2^-8
ACT_SCALE = 32.0

_CACHE = {}

# DVE exp offload: exp(z) = q(z/32)^32, cubic fit of exp on |u| <= 7/32.
EXP_C1 = 1.0000400173833472
EXP_C2 = 0.5014175146307196
EXP_C3 = 0.16555244796209398

# k-tiles whose exp runs on DVE (per parity); chosen away from parity
# boundaries so boundary PVs, O-bank drains, and the S-psum ring are
# never gated on the slower 2-op DVE path.  POOLKT steps run pass A on
# DVE but the 3-squaring pass B on the (otherwise idle) GPSIMD engine —
# slow (~6.4us) but off the critical engines; their PVs are deferred
# ~12 steps.  Early-mid kt only, so the deferred PVs still precede the
# kt=15 stop batch.
OFFKT_P = {0: (1, 4, 7, 10, 13), 1: (1, 4, 7, 10, 13)}
POOLKT = ()


def _register_dve_exp():
    """Register two custom DVE ops (cubic+2 squarings, then 3 squarings).
    TRN2 DVE = v3: 8 ALU stages per pass, so exp needs two chained ops."""
    if "dve_ops" in _CACHE:
        return _CACHE["dve_ops"]
    import concourse.dve_ops as dops
    from concourse.dve_ops import DveOp
    from concourse.dve_spec import Spec, Src0, C0, C1, C2, One, sq
    from concourse.dve_uop import DveOpSpec
    from concourse.dve_spec import lower, _has_src1 as has_src1
    import numpy as np_

    def _ref_expa(in0, in1, c0, c1, c2):
        f = np_.float32
        u = in0.astype(f)
        q = (f(1.0) + u * (f(c0) + u * (f(c1) + u * f(c2)))).astype(f)
        q = (q * q).astype(f)
        return (q * q).astype(f)

    def _ref_expb(in0, in1, c0, c1, c2):
        f = np_.float32
        q = (in0.astype(f) * in0.astype(f)).astype(f)
        q = (q * q).astype(f)
        return (q * q).astype(f)

    body_a = sq(sq(One + Src0 * (C0 + Src0 * (C1 + Src0 * C2))))
    body_b = sq(sq(sq(Src0)))
    spec_a = Spec(body=body_a, reference=_ref_expa)
    spec_b = Spec(body=body_b, reference=_ref_expb)

    ops = []
    for name, spec in (("EXP2A_MHA", spec_a), ("EXP2B_MHA", spec_b)):
        if name in dops._SUB_OPCODE_FOR_NAME:
            op = next(o for o in dops.OPS if o.name == name)
            ops.append(op)
            continue
        row = max(dops._SUB_OPCODE_FOR_NAME.values()) + 1
        assert row < 0x20
        dops._SUB_OPCODE_FOR_NAME[name] = row
        shas = {}
        for ver in ("v3", "v4"):
            try:
                spec_obj = DveOpSpec(name=name, opcode=row,
                                     uops=lower(spec, ver=ver),
                                     rd1_en=has_src1(spec))
                shas[ver] = spec_obj.sha(ver)
            except Exception:
                pass
        op = DveOp(name, spec, subdim=False, uops_sha=shas)
        dops.OPS.append(op)
        dops.CUSTOM_DVE_SPECS[name] = op.spec
        ops.append(op)
    _CACHE["dve_ops"] = ops
    return ops


def _build(reps=1):
    import os
    import concourse.tile as tile
    from concourse import bacc, mybir

    f32 = mybir.dt.float32
    f16 = mybir.dt.float16
    Exp = mybir.ActivationFunctionType.Exp

    dbg_hi = int(os.environ.get("KV2_DEBUG_HI", "-1"))

    nc = bacc.Bacc("TRN2", target_bir_lowering=False, debug=False,
                   num_devices=NCORES)
    q_d = nc.dram_tensor("q", [HPC, N, D], f16, kind="ExternalInput").ap()
    k_d = nc.dram_tensor("k", [HPC, N, D], f16, kind="ExternalInput").ap()
    v_d = nc.dram_tensor("v", [HPC, N, D], f16, kind="ExternalInput").ap()
    o_d = nc.dram_tensor("o", [HPC, N, D], f32, kind="ExternalOutput").ap()
    dbg = {}
    if dbg_hi >= 0:
        for nm, shape, dt_ in (
            ("dbg_qsp", [128, 1024], f16), ("dbg_ksp", [128, 1024], f16),
            ("dbg_ksw", [128, 1024], f16), ("dbg_vaug", [128, 1040], f16),
            ("dbg_pt", [128, 1024], f16), ("dbg_osba", [128, 260], f32),
            ("dbg_osbb", [128, 260], f32),
        ):
            dbg[nm] = nc.dram_tensor(nm, shape, dt_, kind="ExternalOutput").ap()

    expa, expb = _register_dve_exp()

    with tile.TileContext(nc) as tc:
        with (
            tc.tile_pool(name="qsp", bufs=2) as qpool,
            tc.tile_pool(name="ksp", bufs=2) as kpool,
            tc.tile_pool(name="ksw", bufs=2) as wpool,
            tc.tile_pool(name="vap", bufs=3) as vpool,
            tc.tile_pool(name="pt", bufs=10) as ppool,
            tc.tile_pool(name="et", bufs=4) as epool,
            tc.tile_pool(name="pm", bufs=2) as mpool,
            tc.tile_pool(name="osb", bufs=2) as bpool,
            tc.tile_pool(name="ost", bufs=2) as tpool,
            tc.tile_pool(name="spsum", bufs=3, space="PSUM") as spool,
            tc.tile_pool(name="opsum", bufs=1, space="PSUM") as opool,
        ):
            def emit_in_dmas(h, first=False):
                """Issue all input DMAs for head h; returns its tiles.
                q/k transposes are split in halves so the first S matmuls
                (which only need the first 512 columns) start sooner.  For
                the very first head the four transposes are spread across
                the three HWDGE queues (SP/ACT/DVE) to dodge the ~565ns
                per-DMA serial dispatch on a single sequencer."""
                qsp = qpool.tile([128, N // 2], f16, tag="qsp", name="qsp")
                ksp = kpool.tile([128, N // 2], f16, tag="ksp", name="ksp")
                qv = q_d[h].rearrange("(a b) d -> a (b d)", b=2)
                kv = k_d[h].rearrange("(a b) d -> a (b d)", b=2)
                eng_q = nc.sync
                eng_k = nc.sync
                eng_q.dma_start_transpose(qsp[:, 0:512], qv[0:512])
                eng_k.dma_start_transpose(ksp[:, 0:512], kv[0:512])
                eng_q.dma_start_transpose(qsp[:, 512:1024], qv[512:1024])
                eng_k.dma_start_transpose(ksp[:, 512:1024], kv[512:1024])
                # v before ksw: the ksw copies wait on ksp's completion
                # semaphore, and the in-order DMA queue would hold v (needed
                # at the same k-tile) behind that wait
                vaug = vpool.tile([128, NT * 65], f16, tag="vaug", name="vaug")
                v3 = vaug.rearrange("p (t c) -> p t c", c=65)
                src = v_d[h].rearrange("(t p two) d -> two p t d", p=128, two=2)
                nc.sync.dma_start(v3[:, 0:8, 0:64], src[0])
                nc.sync.dma_start(v3[:, 8:16, 0:64], src[1])
                nc.gpsimd.memset(v3[:, :, 64], 1.0)
                ksw = wpool.tile([128, N // 2], f16, tag="ksw", name="ksw")
                nc.sync.dma_start(ksw[0:64, :], ksp[64:128, :])
                nc.sync.dma_start(ksw[64:128, :], ksp[0:64, :])
                if h == dbg_hi:
                    nc.sync.dma_start(dbg["dbg_qsp"], qsp)
                    nc.sync.dma_start(dbg["dbg_ksp"], ksp)
                    nc.sync.dma_start(dbg["dbg_ksw"], ksw)
                    nc.sync.dma_start(dbg["dbg_vaug"], vaug)
                return {"qsp": qsp, "ksp": ksp, "ksw": ksw, "vaug": vaug}

            def lhs_k(t, qh, kt):
                """lhsT [64, 128] for k-tile kt at q-parity qh's range."""
                if kt < 8:   # even k tile
                    if qh == 0:
                        return t["ksp"][0:64, 128 * kt:128 * kt + 128]
                    return t["ksw"][64:128, 128 * kt:128 * kt + 128]
                kk = kt - 8
                if qh == 0:
                    return t["ksw"][0:64, 128 * kk:128 * kk + 128]
                return t["ksp"][64:128, 128 * kk:128 * kk + 128]

            def emit_s(t, st, sq, mid_dve=()):
                qh, kt = divmod(st, NT)
                sT = spool.tile([128, 1024], f32, tag="sT", name="sT")
                w = lhs_k(t, qh, kt)
                q0 = 64 * qh
                for c in range(2):
                    nc.tensor.matmul(
                        sT[:, 512 * c:512 * c + 512], w,
                        t["qsp"][q0:q0 + 64, 512 * c:512 * c + 512],
                        start=True, stop=True)
                pT = ppool.tile([128, 1024], f16, tag="pT", name="pT")
                if kt in OFFKT_P[qh]:
                    et = epool.tile([128, 1024], f16, tag="et", name="et")
                    nc.vector._custom_dve(expa, out=et, in0=sT,
                                          s0=EXP_C1, s1=EXP_C2, imm2=EXP_C3)
                    nc.vector._custom_dve(expb, out=pT, in0=et)
                elif kt in POOLKT:
                    et = epool.tile([128, 1024], f16, tag="et", name="et")
                    nc.vector._custom_dve(expa, out=et, in0=sT,
                                          s0=EXP_C1, s1=EXP_C2, imm2=EXP_C3)
                    m1 = mpool.tile([128, 1024], f16, tag="pm1", name="pm1")
                    nc.gpsimd.tensor_mul(m1, et, et)
                    m2 = mpool.tile([128, 1024], f16, tag="pm2", name="pm2")
                    nc.gpsimd.tensor_mul(m2, m1, m1)
                    nc.gpsimd.tensor_mul(pT, m2, m2)
                else:
                    nc.scalar.activation(pT, sT, Exp, scale=ACT_SCALE)
                if st == 0 and dbg_hi >= 0 and t is tiles.get(dbg_hi):
                    nc.sync.dma_start(dbg["dbg_pt"], pT)
                sq[st] = [pT, 2]

            def emit_pv(t, st, half, sq, octx, dctx, h, tail=False):
                qh, kt = divmod(st, NT)
                key = ("o", half)
                if kt == 0:
                    octx[key] = opool.tile([128, 512], f32,
                                           tag=f"o{half}", name=f"o{half}")
                ob = octx[key]
                ent = sq[st]
                pT = ent[0]
                for j in range(4):
                    qt = 4 * half + j
                    # start=True zeroes the ENTIRE psum bank, so only the
                    # bank's very first matmul may set it; the other slots
                    # accumulate onto the start-cleared bank
                    nc.tensor.matmul(
                        ob[:, 65 * j:65 * j + 65],
                        pT[:, 128 * qt:128 * qt + 128],
                        t["vaug"][:, 65 * kt:65 * kt + 65],
                        start=(kt == 0 and j == 0), stop=(kt == NT - 1))
                ent[1] -= 1
                if ent[1] == 0:
                    del sq[st]

            def emit_drain(octx, h, qh, half, tail_mode=False):
                """Drain one O bank (half 0 on ACT a step after the last PV,
                half 1 on DVE sandwiched in the next kt=1 off-step's exp
                pair; DMA and GPSIMD can't read PSUM), then normalize
                (GPSIMD) and store that half's 4 q-tiles.  Copy shares the
                Exp act table, so no table reload."""
                osb = bpool.tile([128, 260], f32, tag=f"osb{half}",
                                 name=f"osb{half}")
                ob = octx.pop(("o", half))
                if half == 1 and tail_mode:
                    nc.vector.tensor_copy(osb, ob[:, 0:260])
                else:
                    nc.scalar.copy(osb, ob[:, 0:260])
                if h == dbg_hi and qh == 0:
                    nc.sync.dma_start(dbg["dbg_osbb" if half else "dbg_osba"],
                                      osb)
                ost = tpool.tile([128, 256], f32, tag=f"ost{half}",
                                 name=f"ost{half}")
                for j in range(4):
                    nc.gpsimd.normalize_recip(
                        ost[:, 64 * j:64 * j + 64],
                        osb[:, 65 * j:65 * j + 64],
                        osb[:, 65 * j + 64:65 * j + 65])
                dst = o_d[h].rearrange("(t p two) d -> two p t d", p=128, two=2)
                nc.sync.dma_start(dst[qh][:, 4 * half:4 * half + 4, :],
                                  ost.rearrange("p (t d) -> p t d", d=64))

            seq = [i % HPC for i in range(HPC * reps)]

            def handle_pv(gst, phi, pst, phalf):
                emit_pv(tiles[phi], pst, phalf, sqs[phi],
                        octxs.setdefault(phi, {}),
                        dctxs.setdefault(phi, {}), seq[phi])
                if pst % NT == NT - 1:
                    # schedule the bank drain: final parity immediately
                    # (engines idle); half 0 on ACT a step late so the
                    # copy's PV-wait never stalls queued exps; half 1
                    # into the next parity's kt=1 DVE exp sandwich
                    if (phi, pst) in last_par1:
                        dq.append((gst, phi, pst // NT, phalf))
                    else:
                        # both drains on ACT, 1-2 steps late so the copy's
                        # PV-completion wait never stalls queued exp work
                        dq.append((gst + 1 + phalf, phi, pst // NT, phalf))
                if pst == NSTEP - 1 and phalf == 1:
                    # head fully retired; release tile refs
                    tiles.pop(phi - 1, None)

            tiles = {0: emit_in_dmas(seq[0], first=True)}

            # Warm the ACT exp table during the initial DMA fill (after the
            # first head's ACT-queue k DMAs so it doesn't delay them).
            warm = bpool.tile([128, 1], f32, tag="warm", name="warm")
            nc.gpsimd.memset(warm, 0.0)
            warm_o = bpool.tile([128, 1], f16, tag="warmo", name="warmo")
            nc.scalar.activation(warm_o, warm, Exp, scale=1.0)
            pvq = []   # (due_gst, emit_order, hi, st, half)
            sqs = {}   # hi -> {st: [pT, refcount]}
            octxs = {}  # hi -> {("o", half): tile}
            dctxs = {}  # hi -> {(qh, half): osb tile}
            order = 0
            total = len(seq) * NSTEP
            kt0_due = [0, 0]
            max_due = [0, 0]
            dq = []
            sandwich = {}
            last_par1 = {(len(seq) - 1, st) for st in range(NT, NSTEP)}
            gst = 0
            while gst < total + 12:
                # overdue PVs (fractional dues, e.g. the kt=15 batch) must
                # land BEFORE this step's emit_s: the halfB drain is
                # sandwiched inside emit_s and must see the last PV emitted
                while pvq and pvq[0][0] <= gst - 0.5:
                    _, _, phi, pst, phalf = pvq.pop(0)
                    handle_pv(gst, phi, pst, phalf)
                while dq and dq[0][0] <= gst:
                    _, phi, pqh, phalf = dq.pop(0)
                    emit_drain(octxs[phi], seq[phi], pqh, phalf)
                if gst < total:
                    hi, st = divmod(gst, NSTEP)
                    qh, kt = divmod(st, NT)
                    h = seq[hi]
                    sq = sqs.setdefault(hi, {})
                    emit_s(tiles[hi], st, sq,
                           mid_dve=sandwich.pop(gst, ()))
                    if kt == NT - 1:
                        lag_a = lag_b = 1.5
                    elif kt == 0:
                        lag_a, lag_b = 4, 5  # O-bank drain window
                    elif kt in POOLKT:
                        # GPSIMD pass-B latency; parity 1's chain also
                        # queues behind parity 0's on the Pool engine
                        lag_a = lag_b = 12 + 2 * qh
                    elif kt in OFFKT_P[qh]:
                        lag_a = lag_b = 4    # 2-op DVE exp latency
                    else:
                        lag_a = lag_b = 2
                    # clamp: PV emission per half follows kt order (the
                    # kt==0 start=True matmul zeroes the whole bank and the
                    # kt==15 stop batch must be last before the drain)
                    due_a = max(gst + lag_a, max_due[0])
                    due_b = max(gst + lag_b, max_due[1])
                    max_due[0], max_due[1] = due_a, due_b
                    pvq.append((due_a, order, hi, st, 0)); order += 1
                    pvq.append((due_b, order, hi, st, 1)); order += 1
                    pvq.sort()
                    if st == 2 and hi + 1 < len(seq):
                        tiles[hi + 1] = emit_in_dmas(seq[hi + 1])
                while pvq and pvq[0][0] <= gst:
                    _, _, phi, pst, phalf = pvq.pop(0)
                    handle_pv(gst, phi, pst, phalf)
                if gst >= total:
                    while dq:
                        _, phi, pqh, phalf = dq.pop(0)
                        emit_drain(octxs[phi], seq[phi], pqh, phalf,
                                   tail_mode=True)
                gst += 1

    nc.compile()
    return nc


def get_nc(reps=1):
    key = f"nc{reps}"
    if key not in _CACHE:
        _CACHE[key] = _build(reps)
    return _CACHE[key]


def kernel(q, k, v):
    from concourse.bass_utils import run_bass_kernel_spmd

    nc = get_nc()
    # split the 2^-8 prescale as 2^-4 on each operand: exact powers of two,
    # and neither side's values land in the f16 subnormal range
    qf = (np.asarray(q, dtype=np.float32) * np.float32(2.0 ** -4)) \
        .astype(np.float16).reshape(HEADS, N, D)
    kf = (np.asarray(k, dtype=np.float32) * np.float32(2.0 ** -4)) \
        .astype(np.float16).reshape(HEADS, N, D)
    vf = np.asarray(v, dtype=np.float32).astype(np.float16).reshape(HEADS, N, D)
    in_maps = [
        {
            "q": np.ascontiguousarray(qf[c * HPC:(c + 1) * HPC]),
            "k": np.ascontiguousarray(kf[c * HPC:(c + 1) * HPC]),
            "v": np.ascontiguousarray(vf[c * HPC:(c + 1) * HPC]),
        }
        for c in range(NCORES)
    ]
    res = run_bass_kernel_spmd(nc, in_maps, list(range(NCORES)))
    out = np.concatenate([res.results[c]["o"] for c in range(NCORES)], axis=0)
    return np.ascontiguousarray(out.reshape(B, H, N, D).astype(np.float32))

